# revision 19
# baseline (speedup 1.0000x reference)
"""Distributed GQA attention (B=2,T=2048,C=2048,H=16,KV=4,D=128, RoPE, causal)
for one TRN2 chip (8 NeuronCores).

Sharding (no collectives): core c -> batch b=c//4, stripe s=c%4.
Each core handles query rows {r : r % 4 == s} of its batch (512 rows,
interleaved so causal spans are shape-uniform across cores -> one SPMD graph),
computes K/V for the full sequence (replicated KV projection), and produces
complete output rows. Host reassembles by stripe.

Per-core pipeline:
  KVproj (bf16 x^T resident)  -> K^T[d,t] (+RoPE, f32r), V[t,d] (bf16)
  Qproj  (f32r)               -> Q^T[d,(h,q)] (+RoPE/sqrt(D), f32r)
  attention, scores transposed: S^T[k,(h4,q)] = K̂-tile^T · Q̂(4 heads of group)
    + staircase causal mask (host data), exp on ACT -> P^T bf16,
    denominators via ones-vector matmul, AV: Y^T = V_g^T · P^T,
    normalization via reciprocal + gpsimd partition_broadcast
  Oproj  (f32r), out rows DMA'd per 128x512 tile.
"""

import numpy as np
import ml_dtypes

import concourse.bass as bass
import concourse.tile as tile
from concourse import bacc, mybir
from concourse.bass_utils import run_bass_kernel_spmd

B, T, C = 2, 2048, 2048
H, KV, D = 16, 4, 128
G4 = H // KV            # q heads per kv head
THETA = 10000.0
P = 128
CT = C // P             # 16 c-tiles
TQ = 512                # queries per core
NQT = TQ // P           # 4 q-tiles
NTT = T // P            # 16 token tiles
MASK_VAL = -1e5

f32 = mybir.dt.float32
f32r = mybir.dt.float32r
bf16 = mybir.dt.bfloat16

_compiled = {}


def _build():
    nc = bacc.Bacc("TRN2", target_bir_lowering=False, debug=False, num_devices=8)
    xq_e = nc.dram_tensor("xq", [P, CT * TQ], bf16, kind="ExternalInput")
    xkv_e = nc.dram_tensor("xkv", [P, CT * TQ], bf16, kind="ExternalInput")  # chunk, pre-tiled [p,(ct t)]
    wq_e = nc.dram_tensor("wq", [P, H * CT * D], bf16, kind="ExternalInput")  # [p,(h ct d)]
    wkv_e = nc.dram_tensor("wkv", [P, CT * 2 * KV * D], bf16, kind="ExternalInput")  # [p,(ct n)]
    wo_e = nc.dram_tensor("wo", [P, (C // 512) * H * 512], bf16, kind="ExternalInput")  # [p,(cc hh c)]
    cq_e = nc.dram_tensor("cos_q", [D, TQ], f32, kind="ExternalInput")
    sq_e = nc.dram_tensor("sin_q", [D, TQ], f32, kind="ExternalInput")
    ck_e = nc.dram_tensor("cos_k", [D, TQ], bf16, kind="ExternalInput")  # chunk positions
    sk_e = nc.dram_tensor("sin_k", [D, TQ], bf16, kind="ExternalInput")
    mk_e = nc.dram_tensor("mask", [P, NQT * P], bf16, kind="ExternalInput")
    out_e = nc.dram_tensor("out", [TQ, C], f32, kind="ExternalOutput")

    NR = 4  # ranks per batch group

    from contextlib import ExitStack

    with tile.TileContext(nc) as tc, ExitStack() as top:
        persist = top.enter_context(tc.tile_pool(name="persist", bufs=1))

        mask_t = persist.tile([P, NQT, P], bf16)
        nc.sync.dma_start(mask_t[:], mk_e.ap().rearrange("p (kt q) -> p kt q", kt=NQT))
        ones_col = persist.tile([P, 1], bf16)
        nc.vector.memset(ones_col[:], 1.0)
        NRk = 4
        qhat = persist.tile([D, H, TQ], bf16)
        khat = persist.tile([D, NRk, KV, TQ], bf16)
        vsb = persist.tile([P, NTT, KV * D], bf16)

        # Q operand pools (DMAs emitted after the KV-chunk matmuls so the KV
        # inputs get DMA bandwidth first).
        qstack = ExitStack()
        xqp = qstack.enter_context(tc.tile_pool(name="xqp", bufs=1))
        wstream = qstack.enter_context(tc.tile_pool(name="wqstream", bufs=8))
        tabq = qstack.enter_context(tc.tile_pool(name="tabq", bufs=1))

        # ---- KV chunk projection (bf16) + AllGather ---------------------
        with tc.tile_pool(name="kvchunk", bufs=1) as kvc, \
             tc.tile_pool(name="dram", bufs=1, space="DRAM") as dram, \
             tc.tile_pool(name="ps_kvp", bufs=3, space="PSUM") as ps_kv, \
             tc.tile_pool(name="ropek", bufs=2) as ropekp:
            xkv = kvc.tile([P, CT, TQ], bf16)
            nc.sync.dma_start(xkv[:], xkv_e.ap().rearrange("p (ct t) -> p ct t", ct=CT))
            wkv = kvc.tile([P, CT, 2 * KV * D], bf16)
            wkv_r = wkv_e.ap().rearrange("p (ct n) -> p ct n", ct=CT)
            nc.sync.dma_start(wkv[:, :, 0:KV * D], wkv_r[:, :, 0:KV * D])
            cos_k = kvc.tile([D, TQ], bf16)
            nc.sync.dma_start(cos_k[:], ck_e.ap())
            sin_k = kvc.tile([D, TQ], bf16)
            nc.sync.dma_start(sin_k[:], sk_e.ap())

            kchunk = kvc.tile([D, KV, TQ], bf16)
            vchunk = kvc.tile([P, NQT, KV * D], bf16)

            for g in range(KV):
                ps = ps_kv.tile([P, TQ], f32, tag="ps_kv")
                for ct in range(CT):
                    nc.tensor.matmul(ps[:], wkv[:, ct, g * D:(g + 1) * D],
                                     xkv[:, ct, :],
                                     start=(ct == 0), stop=(ct == CT - 1))
                tmp = ropekp.tile([D, TQ], f32, tag="rope_k")
                nc.vector.tensor_copy(tmp[0:64, :], ps[64:128, :])
                nc.vector.tensor_copy(tmp[64:128, :], ps[0:64, :])
                ksl = kchunk[:, g, :]
                nc.vector.tensor_mul(ksl, ps[:], cos_k[:])
                nc.vector.tensor_mul(tmp[:], tmp[:], sin_k[:])
                nc.vector.tensor_add(ksl, ksl, tmp[:])

            nc.sync.dma_start(wkv[:, :, KV * D:2 * KV * D], wkv_r[:, :, KV * D:2 * KV * D])
            xq = xqp.tile([P, CT, TQ], bf16)
            nc.sync.dma_start(xq[:], xq_e.ap().rearrange("p (ct q) -> p ct q", ct=CT))
            cos_q = tabq.tile([D, TQ], f32)
            nc.sync.dma_start(cos_q[:], cq_e.ap())
            sin_q = tabq.tile([D, TQ], f32)
            nc.sync.dma_start(sin_q[:], sq_e.ap())

            for ttl in range(NQT):
                ps = ps_kv.tile([P, KV * D], f32, tag="ps_kv")
                for ct in range(CT):
                    nc.tensor.matmul(ps[:], xkv[:, ct, ttl * P:(ttl + 1) * P],
                                     wkv[:, ct, KV * D:2 * KV * D],
                                     start=(ct == 0), stop=(ct == CT - 1))
                nc.vector.tensor_copy(vchunk[:, ttl, :], ps[:])

            cc_in = dram.tile([2, P, KV, TQ], bf16)
            cc_out = dram.tile([NR, 2, P, KV, TQ], bf16)
            nc.sync.dma_start(cc_in[0], kchunk[:])
            nc.sync.dma_start(cc_in[1], vchunk[:].rearrange("p t n -> p (t n)").rearrange("p (g x) -> p g x", g=KV))
            nc.gpsimd.collective_compute(
                "AllGather",
                mybir.AluOpType.bypass,
                replica_groups=[[0, 1, 2, 3], [4, 5, 6, 7]],
                ins=[cc_in[:].opt()],
                outs=[cc_out[:].opt()],
            )
            # khat[d, r, g, t] <- cc_out[r, 0, d, g, t]: contiguous 2KB runs
            nc.sync.dma_start(
                khat[:], cc_out[:, 0].rearrange("r d g t -> d r g t"))
            # vsb[p, (r ttl), n] <- cc_out[r, 1, p, ttl, n]
            nc.sync.dma_start(
                vsb[:].rearrange("p (r ttl) n -> p r ttl n", r=NR),
                cc_out[:, 1].rearrange("r p g x -> p r (g x)").rearrange("p r (ttl n) -> p r ttl n", ttl=NQT))

        # ---- Q projection (bf16) ----------------------------------------
        with tc.tile_pool(name="ps_qp", bufs=3, space="PSUM") as ps_q, \
             tc.tile_pool(name="ropeq", bufs=2) as ropep:
            for h in range(H):
                wqt = wstream.tile([P, CT, D], bf16, tag="wq")
                nc.sync.dma_start(
                    wqt[:], wq_e.ap().rearrange("p (h ct d) -> p h ct d", h=H, ct=CT)[:, h])
                ps = ps_q.tile([P, TQ], f32, tag="ps_q")
                for ct in range(CT):
                    nc.tensor.matmul(ps[:], wqt[:, ct, :], xq[:, ct, :],
                                     start=(ct == 0), stop=(ct == CT - 1))
                tmp = ropep.tile([D, TQ], f32, tag="rope_q")
                nc.vector.tensor_copy(tmp[0:64, :], ps[64:128, :])
                nc.vector.tensor_copy(tmp[64:128, :], ps[0:64, :])
                qsl = qhat[:, h, :]
                nc.vector.tensor_mul(qsl, ps[:], cos_q[:])
                nc.vector.tensor_mul(tmp[:], tmp[:], sin_q[:])
                nc.vector.tensor_add(qsl, qsl, tmp[:])
        qstack.close()

        # ---- attention (scores transposed) ------------------------------
        yhat = persist.tile([D, H, TQ], bf16)
        wostream = top.enter_context(tc.tile_pool(name="wostream", bufs=2))
        with tc.tile_pool(name="ptile", bufs=10) as ptp, \
             tc.tile_pool(name="small", bufs=3) as small, \
             tc.tile_pool(name="ps_s", bufs=2, space="PSUM") as ps_sp, \
             tc.tile_pool(name="ps_y", bufs=2, space="PSUM") as ps_yp, \
             tc.tile_pool(name="ps_den", bufs=1, space="PSUM") as ps_denp:
            for qt in range(NQT):
                nkt = 4 * (qt + 1)
                npair = nkt // 2
                for g in range(KV):
                    ps_y = ps_yp.tile([P, G4, P], f32, tag="ps_y")
                    ps_den = ps_denp.tile([1, G4 * P], f32, tag="ps_den")

                    def emit_scores(pb):
                        ps_s = ps_sp.tile([P, 2, G4, P], f32, tag="ps_s", name=f"ps_s{pb}")
                        for j in range(2):
                            kt = 2 * pb + j
                            nc.tensor.matmul(ps_s[:, j], khat[:, kt // 4, g, (kt % 4) * P:(kt % 4 + 1) * P],
                                             qhat[:, g * G4:(g + 1) * G4, qt * P:(qt + 1) * P],
                                             start=True, stop=True)
                        if 2 * pb >= nkt - 4:
                            ktl = 2 * pb - (nkt - 4)
                            nc.vector.tensor_add(
                                ps_s[:], ps_s[:],
                                mask_t[:, ktl:ktl + 2, None, :].to_broadcast((P, 2, G4, P)))
                        pt = ptp.tile([P, 2, G4, P], bf16, tag="pt", name=f"pt{pb}")
                        nc.scalar.activation(pt[:], ps_s[:], mybir.ActivationFunctionType.Exp)
                        return pt

                    def emit_av(pb, pt):
                        for j in range(2):
                            kt = 2 * pb + j
                            ptf = pt[:, j].rearrange("p h q -> p (h q)")
                            nc.tensor.matmul(ps_den[:], ones_col[:], ptf,
                                             start=(kt == 0), stop=(kt == nkt - 1))
                            nc.tensor.matmul(ps_y[:], vsb[:, kt, g * D:(g + 1) * D], ptf,
                                             start=(kt == 0), stop=(kt == nkt - 1))

                    pend = None
                    for pb in range(npair):
                        blk = (pb, emit_scores(pb))
                        if pend is not None:
                            emit_av(*pend)
                        pend = blk
                    emit_av(*pend)
                    den = small.tile([1, G4 * P], f32, tag="den")
                    nc.vector.tensor_copy(den[:], ps_den[:])
                    rec = small.tile([1, G4 * P], f32, tag="rec")
                    nc.vector.reciprocal_approx_fast(rec[:], den[:])
                    bc = small.tile([P, G4, P], f32, tag="bc")
                    nc.gpsimd.partition_broadcast(bc[:], rec[:])
                    ysl = yhat[:, g * G4:(g + 1) * G4, qt * P:(qt + 1) * P]
                    nc.vector.tensor_mul(ysl, ps_y[:], bc[:])

        # ---- output projection (f32r) -----------------------------------
        with tc.tile_pool(name="outp", bufs=3) as outp, \
             tc.tile_pool(name="ps_o", bufs=2, space="PSUM") as ps_op:
            for cc in range(C // 512):
                wot = wostream.tile([P, H, 512], bf16, tag="wo")
                nc.sync.dma_start(
                    wot[:], wo_e.ap().rearrange("p (cc hh c) -> p cc hh c", cc=C // 512, hh=H)[:, cc])
                ps_os = [ps_op.tile([P, 512], f32, tag=f"ps_o{qt}", name=f"ps_o{qt}")
                         for qt in range(NQT)]
                for hh in range(H):
                    for qt in range(NQT):
                        nc.tensor.matmul(ps_os[qt][:], yhat[:, hh, qt * P:(qt + 1) * P], wot[:, hh, :],
                                         start=(hh == 0), stop=(hh == H - 1))
                for qt in range(NQT):
                    osb = outp.tile([P, 512], f32, tag="osb")
                    nc.vector.tensor_copy(osb[:], ps_os[qt][:])
                    nc.sync.dma_start(out_e.ap()[qt * P:(qt + 1) * P, cc * 512:(cc + 1) * 512], osb[:])

    nc.compile()
    return nc


def _rope_tables():
    freqs = 1.0 / (THETA ** (np.arange(0, D, 2, dtype=np.float64) / D))
    ang = np.arange(T, dtype=np.float64)[:, None] * freqs[None, :]
    emb = np.concatenate([ang, ang], axis=-1)          # [T, D]
    return np.cos(emb), np.sin(emb)                    # [T, D] each


def _prep_inputs(x, Wq, Wkv, Wo):
    cos, sin = _rope_tables()
    sgn = np.where(np.arange(D) < D // 2, -1.0, 1.0)   # sign for shifted term
    inv = 1.0 / np.sqrt(D)
    cosT = np.ascontiguousarray(cos.T)                 # [D, T]
    sinTs = np.ascontiguousarray(sin.T) * sgn[:, None]

    # pre-tiled layouts: every DMA reads contiguous per-partition runs
    # wq [p, (h ct d)]: wq[p, h, ct, d] = Wq.T[ct*128+p, h*128+d]
    wq_t = np.ascontiguousarray(
        Wq.T.reshape(16, 128, 16, 128).transpose(1, 2, 0, 3).reshape(128, -1)
    ).astype(ml_dtypes.bfloat16)
    # wkv [p, (ct n)]: wkv[p, ct, n] = Wkv.T[ct*128+p, n]
    wkv_t = np.ascontiguousarray(
        Wkv.T.reshape(16, 128, 1024).transpose(1, 0, 2).reshape(128, -1)
    ).astype(ml_dtypes.bfloat16)
    # wo [p, (cc hh c)]: wo[p, cc, hh, c] = Wo.T[hh*128+p, cc*512+c]
    wo_t = np.ascontiguousarray(
        Wo.T.reshape(16, 128, 4, 512).transpose(1, 2, 0, 3).reshape(128, -1)
    ).astype(ml_dtypes.bfloat16)

    in_maps = []
    for c in range(8):
        b, s = c // 4, c % 4
        rows = np.arange(s, T, 4)
        xq = np.ascontiguousarray(
            x[b][rows, :].T.reshape(16, 128, 512).transpose(1, 0, 2).reshape(128, -1)
        ).astype(ml_dtypes.bfloat16)  # [p, (ct q)]
        ch = np.arange(512 * s, 512 * (s + 1))
        xkv = np.ascontiguousarray(
            x[b][ch, :].T.reshape(16, 128, 512).transpose(1, 0, 2).reshape(128, -1)
        ).astype(ml_dtypes.bfloat16)  # [p, (ct t)] chunk
        cq = np.ascontiguousarray(cosT[:, rows] * inv, dtype=np.float32)
        sq = np.ascontiguousarray(sinTs[:, rows] * inv, dtype=np.float32)
        # staircase mask, transposed: [k-window j, q i]; visible iff j <= 4i+s
        j = np.arange(TQ)[:, None]
        i = np.arange(P)[None, :]
        mask = np.where(j <= 4 * i + s, 0.0, MASK_VAL).astype(np.float32)
        # pre-tiled [p, (kt q)]: mask_t[p, kt, q] = mask[kt*128+p, q]
        mask = np.ascontiguousarray(
            mask.reshape(4, 128, 128).transpose(1, 0, 2).reshape(128, -1)
        ).astype(ml_dtypes.bfloat16)
        in_maps.append({
            "xq": xq, "xkv": xkv,
            "wq": wq_t, "wkv": wkv_t, "wo": wo_t,
            "cos_q": cq, "sin_q": sq,
            "cos_k": np.ascontiguousarray(cosT[:, ch]).astype(ml_dtypes.bfloat16),
            "sin_k": np.ascontiguousarray(sinTs[:, ch]).astype(ml_dtypes.bfloat16),
            "mask": mask,
        })
    return in_maps


def _unshard(results):
    full = np.empty((B, T, C), dtype=np.float32)
    for c in range(8):
        b, s = c // 4, c % 4
        full[b, s::4, :] = results[c]["out"]
    return full


def run(x, Wq, Wkv, Wo, trace=False, trace_kwargs=None):
    if "nc" not in _compiled:
        _compiled["nc"] = _build()
    nc = _compiled["nc"]
    in_maps = _prep_inputs(np.asarray(x), np.asarray(Wq), np.asarray(Wkv), np.asarray(Wo))
    res = run_bass_kernel_spmd(nc, in_maps, core_ids=list(range(8)), trace=trace,
                               **(trace_kwargs or {}))
    return _unshard(res.results), res


def kernel(x, Wq, Wkv, Wo):
    out, _ = run(x, Wq, Wkv, Wo, trace=False)
    return out
